# revision 11
# baseline (speedup 1.0000x reference)
"""GAT (2-layer graph attention network + output MLP) on 8 Trainium2 NeuronCores.

Strategy ("diagonal scheduling"):
  - The Bass program is built per-invocation, so the graph structure is a
    compile-time constant.  Nodes are assigned to cores balancing total
    in-degree, and within each core nodes are grouped into 128-node blocks
    sorted by (deg_lo, deg_hi) so that all nodes in a block have nearly equal
    in-degree from each half of the node space.
  - Edges of a block are laid out in "chunks" of 128 slots: slot (c, p) holds
    the c-th in-edge of the node on partition p.  A chunk therefore has at
    most one edge per destination, which turns the segment-softmax scatter
    into a plain PSUM accumulation with a constant identity stationary matrix
    (no masks, no segment ops).
  - Per layer, each core computes the feature/attention table rows for its own
    nodes ([h | alpha_src] per node), all-gathers the full table to DRAM, and
    then gathers per-edge rows with dma_gather (int16 indices force a lo/hi
    table split at NTOT/2).
  - softmax uses exp(leakyrelu(e)) = max(exp(e), exp(0.2 e)) and folds the
    1/z normalization after aggregation (exact same math as the reference,
    max-subtraction is skipped since logits are O(1)).

kernel(**inputs) -> np.ndarray  takes full inputs, returns the full output.
"""

import numpy as np

# ---------------------------------------------------------------- constants
N, E, F_IN, D_HID, H, N_CLS = 50000, 800000, 128, 96, 8, 40
DH = D_HID // H  # 12
NEG_SLOPE = 0.2
CORES = 8
BLK = 128
PAD_AS = -10000.0  # alpha_src for pad rows: exp(0.2*(PAD_AS+ad)) == 0.0

_CACHE = {}


# ---------------------------------------------------------------- planning
def make_plan(edge_src, edge_dst, n=N, cores=CORES, blk=BLK):
    """Pure graph-structure planning (numpy only).

    Returns a dict with the node permutation, per-block common chunk counts
    and the per-core wrapped int16 gather-index arrays.
    """
    edge_src = np.asarray(edge_src, dtype=np.int64)
    edge_dst = np.asarray(edge_dst, dtype=np.int64)
    e = len(edge_src)

    deg = np.bincount(edge_dst, minlength=n)

    # nodes per core, including dummies; one dummy pinned last on every core
    npc = -(-(n + cores) // (cores * blk)) * blk  # round up to block multiple
    ntot = cores * npc
    half = ntot // 2

    # --- assign real nodes to cores balancing total degree (snake deal) ---
    order = np.argsort(-deg, kind="stable")  # real nodes by degree desc
    core_of = np.empty(n, dtype=np.int64)
    # snake pattern over rounds of 2*cores
    r = np.arange(n)
    rnd = r // cores
    pos = r % cores
    fwd = (rnd % 2) == 0
    lane = np.where(fwd, pos, cores - 1 - pos)
    core_of[order] = lane

    # lo set = cores 0..cores/2-1
    is_lo_node = core_of < (cores // 2)
    src_is_lo = is_lo_node[edge_src]
    d_lo = np.bincount(edge_dst[src_is_lo], minlength=n)
    d_hi = deg - d_lo

    # --- per-half global ordering, dealt round-robin to the half's cores ---
    # Sorting each half globally by (d_lo desc, d_hi snake) and dealing node
    # at sorted position g to core g%hc, slot g//hc keeps every core's block
    # profile an interleaved sample of the same distribution, so the common
    # (cross-core max) chunk counts stay tight.  Dummies sort last, which
    # pins one dummy at the final slot of every core (used as the pad row).
    new_of_old = np.empty(n, dtype=np.int64)
    old_of_new = np.full(ntot, -1, dtype=np.int64)
    hc = cores // 2
    for side in (0, 1):
        mine = np.where(is_lo_node == (side == 0))[0]
        dl, dhh = d_lo[mine], d_hi[mine]
        run_parity = (dl.max() - dl) % 2  # alternate d_hi dir per d_lo run
        key_hi = np.where(run_parity == 0, -dhh, dhh)
        srt = mine[np.lexsort((key_hi, -dl))]
        assert len(srt) <= hc * npc - hc, (len(srt), npc)
        g = np.arange(len(srt))
        core = side * hc + g % hc
        slot = g // hc
        newids = core * npc + slot
        new_of_old[srt] = newids
        old_of_new[newids] = srt

    nblk = npc // blk  # blocks per core

    # --- per (core, block) lo/hi chunk counts -> common across cores ---
    d_lo_new = np.zeros(ntot, dtype=np.int64)
    d_hi_new = np.zeros(ntot, dtype=np.int64)
    real = old_of_new >= 0
    d_lo_new[real] = d_lo[old_of_new[real]]
    d_hi_new[real] = d_hi[old_of_new[real]]
    # block max per core
    kl_cb = d_lo_new.reshape(cores, nblk, blk).max(axis=2)
    kh_cb = d_hi_new.reshape(cores, nblk, blk).max(axis=2)
    K_lo = kl_cb.max(axis=0)  # [nblk] common
    K_hi = kh_cb.max(axis=0)

    off_lo = np.concatenate([[0], np.cumsum(K_lo * blk)])  # slot offsets
    off_hi = np.concatenate([[0], np.cumsum(K_hi * blk)])
    S_lo = int(off_lo[-1])
    S_hi = int(off_hi[-1])

    pad_lo = half - 1  # last node of core cores/2-1 (pinned dummy)
    pad_hi = half - 1  # (value in hi-table local coords: ntot-1-half)

    # --- slot filling ---
    dst_new = new_of_old[edge_dst]
    src_new = new_of_old[edge_src]
    is_lo = src_new < half

    # rank of each edge within its (dst, class) group
    # sort edges by (class, dst_new) then rank = position - group start
    grp = dst_new * 2 + (~is_lo)  # group id
    srt = np.argsort(grp, kind="stable")
    grp_s = grp[srt]
    starts = np.concatenate([[0], np.where(np.diff(grp_s) != 0)[0] + 1])
    group_start = np.zeros(len(grp_s), dtype=np.int64)
    group_start[starts] = starts
    group_start = np.maximum.accumulate(group_start)
    rank_s = np.arange(e) - group_start
    rank = np.empty(e, dtype=np.int64)
    rank[srt] = rank_s

    core_e = dst_new // npc
    blk_e = (dst_new % npc) // blk
    p_e = dst_new % blk

    slots_lo = np.full((cores, S_lo), pad_lo, dtype=np.int16)
    slots_hi = np.full((cores, S_hi), pad_hi, dtype=np.int16)

    lo_m = is_lo
    pos_lo = off_lo[blk_e[lo_m]] + rank[lo_m] * blk + p_e[lo_m]
    slots_lo[core_e[lo_m], pos_lo] = src_new[lo_m].astype(np.int16)
    hi_m = ~is_lo
    pos_hi = off_hi[blk_e[hi_m]] + rank[hi_m] * blk + p_e[hi_m]
    slots_hi[core_e[hi_m], pos_hi] = (src_new[hi_m] - half).astype(np.int16)

    # wrap for dma_gather: element i -> [i%16, i//16], tiled to 128 partitions
    def wrap(a):
        # a: [cores, S] -> [cores, 128, S//16]
        s = a.shape[1]
        if s == 0:
            return np.zeros((cores, 128, 0), dtype=np.int16)
        w = a.reshape(cores, s // 16, 16).transpose(0, 2, 1)  # [cores,16,S/16]
        return np.ascontiguousarray(np.tile(w, (1, 8, 1)))

    return dict(
        n=n, e=e, cores=cores, npc=npc, ntot=ntot, half=half, nblk=nblk,
        new_of_old=new_of_old, old_of_new=old_of_new,
        K_lo=K_lo.astype(np.int64), K_hi=K_hi.astype(np.int64),
        off_lo=off_lo, off_hi=off_hi, S_lo=S_lo, S_hi=S_hi,
        idx_lo=wrap(slots_lo), idx_hi=wrap(slots_hi),
        util=float(e) / max(1.0, float((S_lo + S_hi) * cores)),
    )


# ---------------------------------------------------------------- program
def build_program(plan, f_in=F_IN, d_hid=D_HID, h=H, n_cls=N_CLS, bf16=False,
                  stop_after=None):
    import concourse.bacc as bacc
    import concourse.mybir as mybir
    from concourse import tile

    dt = mybir.dt
    f32 = dt.float32
    TDT = dt.bfloat16 if bf16 else dt.float32
    dh = d_hid // h
    npc, nblk, half = plan["npc"], plan["nblk"], plan["half"]
    K_lo, K_hi = plan["K_lo"], plan["K_hi"]
    off_lo, off_hi = plan["off_lo"], plan["off_hi"]
    S_lo, S_hi = plan["S_lo"], plan["S_hi"]
    cores = plan["cores"]
    ntot = plan["ntot"]
    ROW = 128  # table row, elements (512B fp32 / 256B bf16)
    DCAT = d_hid + 2 * h  # 112

    nc = bacc.Bacc("TRN2", target_bir_lowering=False, debug=False,
                   num_devices=cores)

    # ---- I/O ----
    xT = nc.dram_tensor("xT", [f_in, npc], f32, kind="ExternalInput")
    W0cat = nc.dram_tensor("W0cat", [f_in, DCAT], f32, kind="ExternalInput")
    W1cat = nc.dram_tensor("W1cat", [d_hid, DCAT], f32, kind="ExternalInput")
    Wout = nc.dram_tensor("Wout", [d_hid, n_cls], f32, kind="ExternalInput")
    b0b = nc.dram_tensor("b0b", [128, d_hid], f32, kind="ExternalInput")
    b1b = nc.dram_tensor("b1b", [128, d_hid], f32, kind="ExternalInput")
    boutb = nc.dram_tensor("boutb", [128, n_cls], f32, kind="ExternalInput")
    identt = nc.dram_tensor("identt", [128, 128], TDT, kind="ExternalInput")
    ident32 = nc.dram_tensor("ident32", [128, 128], f32, kind="ExternalInput")
    idx_lo_d = nc.dram_tensor("idx_lo", [128, max(S_lo // 16, 1)], dt.int16,
                              kind="ExternalInput")
    idx_hi_d = nc.dram_tensor("idx_hi", [128, max(S_hi // 16, 1)], dt.int16,
                              kind="ExternalInput")
    out_d = nc.dram_tensor("out", [npc, n_cls], f32, kind="ExternalOutput")

    Kmax = int(max(1, (K_lo + K_hi).max()))

    with tile.TileContext(nc) as tc:
        with (
            tc.tile_pool(name="dram", bufs=1, space="DRAM") as dramp,
            tc.tile_pool(name="persist", bufs=1) as pers,
            tc.tile_pool(name="gath", bufs=3) as gath,
            tc.tile_pool(name="stage", bufs=3) as stage,
            tc.tile_pool(name="small", bufs=4) as small,
            tc.tile_pool(name="psA", bufs=2, space="PSUM") as psA,
            tc.tile_pool(name="psB", bufs=2, space="PSUM") as psB,
            tc.tile_pool(name="psT", bufs=2, space="PSUM") as psT,
        ):
            # ---- DRAM scratch ----
            tslice = dramp.tile([npc, ROW], TDT)
            tfull0 = dramp.tile([ntot, ROW], TDT, addr_space="Shared")
            tfull1 = dramp.tile([ntot, ROW], TDT, addr_space="Shared")

            # ---- persistent SBUF ----
            xT_sb = pers.tile([f_in, npc], f32)
            nc.sync.dma_start(xT_sb[:], xT[:, :])
            W0_sb = pers.tile([f_in, DCAT], f32)
            nc.sync.dma_start(W0_sb[:], W0cat[:, :])
            W1_sb = pers.tile([d_hid, DCAT], f32)
            nc.sync.dma_start(W1_sb[:], W1cat[:, :])
            Wo_sb = pers.tile([d_hid, n_cls], f32)
            nc.sync.dma_start(Wo_sb[:], Wout[:, :])
            b0_sb = pers.tile([128, d_hid], f32)
            nc.sync.dma_start(b0_sb[:], b0b[:, :])
            b1_sb = pers.tile([128, d_hid], f32)
            nc.sync.dma_start(b1_sb[:], b1b[:, :])
            bo_sb = pers.tile([128, n_cls], f32)
            nc.sync.dma_start(bo_sb[:], boutb[:, :])
            idt_sb = pers.tile([128, 128], TDT)
            nc.sync.dma_start(idt_sb[:], identt[:, :])
            id32_sb = pers.tile([128, 128], f32)
            nc.sync.dma_start(id32_sb[:], ident32[:, :])
            if S_lo:
                ixlo_sb = pers.tile([128, S_lo // 16], dt.int16)
                nc.sync.dma_start(ixlo_sb[:], idx_lo_d[:, :])
            if S_hi:
                ixhi_sb = pers.tile([128, S_hi // 16], dt.int16)
                nc.sync.dma_start(ixhi_sb[:], idx_hi_d[:, :])
            ad0_sb = pers.tile([128, nblk * h], TDT)
            ad1_sb = pers.tile([128, nblk * h], TDT)
            h1_sb = pers.tile([128, nblk * d_hid], f32)
            h2_sb = pers.tile([128, nblk * d_hid], f32)
            padrow = pers.tile([1, h], TDT)
            nc.vector.memset(padrow[:], PAD_AS)

            # ================= helper: table build =================
            def table_build(src_lhsT, Wc_sb, ad_sb, tf):
                """src_lhsT(b) -> lhsT AP [k, 128] for block b."""
                for b in range(nblk):
                    lhsT = src_lhsT(b)
                    ps = psA.tile([128, DCAT], mybir.dt.float32)
                    nc.tensor.matmul(ps[:], lhsT, Wc_sb[:, :], start=True,
                                     stop=True)
                    stg = stage.tile([128, ROW], TDT, tag="stg")
                    nc.vector.tensor_copy(stg[:, 0:DCAT], ps[:, 0:DCAT])
                    nc.vector.memset(stg[:, DCAT:ROW], 0.0)
                    nc.vector.tensor_copy(
                        ad_sb[:, b * h:(b + 1) * h],
                        ps[:, d_hid + h:d_hid + 2 * h])
                    nc.sync.dma_start(
                        tslice[b * 128:(b + 1) * 128, :], stg[:])
                # pad row: overwrite alpha_src of the core's last node
                nc.sync.dma_start(
                    tslice[npc - 1:npc, d_hid:d_hid + h], padrow[:])
                nc.gpsimd.collective_compute(
                    "AllGather", mybir.AluOpType.bypass,
                    replica_groups=[list(range(cores))],
                    ins=[tslice[:, :]], outs=[tf[:, :]])

            # ================= helper: edge phase =================
            def edge_phase(tf, ad_sb, post):
                """post(b, ps) consumes psum [128, d_hid+h] for block b."""
                for b in range(nblk):
                    kl, kh = int(K_lo[b]), int(K_hi[b])
                    K = kl + kh
                    if K == 0:
                        post(b, None)
                        continue
                    G = gath.tile([128, Kmax * 128], TDT, tag="G")
                    if kl:
                        nc.gpsimd.dma_gather(
                            G[:, :kl * 128].rearrange("p (k e) -> p k e",
                                                      e=128),
                            tf[0:half, :],
                            ixlo_sb[:, off_lo[b] // 16:off_lo[b + 1] // 16],
                            128 * kl, 128 * kl, ROW,
                            single_packet=False)
                    if kh:
                        nc.gpsimd.dma_gather(
                            G[:, kl * 128:K * 128].rearrange(
                                "p (k e) -> p k e", e=128),
                            tf[half:ntot, :],
                            ixhi_sb[:, off_hi[b] // 16:off_hi[b + 1] // 16],
                            128 * kh, 128 * kh, ROW,
                            single_packet=False)
                    Gv = G[:, :K * 128].rearrange("p (k e) -> p k e", e=128)
                    asv = Gv[:, :, d_hid:d_hid + h]
                    ev = Gv[:, :, 112:120]
                    uv = Gv[:, :, 120:128]
                    adb = ad_sb[:, b * h:(b + 1) * h].rearrange(
                        "p (o j) -> p o j", o=1).broadcast_to([128, K, h])
                    # e = alpha_src + alpha_dst
                    nc.vector.tensor_add(ev, asv, adb)
                    # s_exp = max(exp(e), exp(0.2 e))  == exp(leakyrelu(e))
                    nc.scalar.activation(uv, ev,
                                         mybir.ActivationFunctionType.Exp)
                    nc.scalar.activation(asv, ev,
                                         mybir.ActivationFunctionType.Exp,
                                         scale=NEG_SLOPE)
                    nc.vector.tensor_max(asv, asv, uv)
                    # msg = h_src * s_exp  (in place over h columns)
                    msgv = Gv[:, :, 0:d_hid].rearrange(
                        "p k (j d) -> p k j d", d=dh)
                    sexp = asv.rearrange("p k (j o) -> p k j o",
                                         o=1).broadcast_to([128, K, h, dh])
                    nc.vector.tensor_mul(msgv, msgv, sexp)
                    # accumulate [msg | s_exp] into psum via identity matmul
                    ps = psB.tile([128, d_hid + h], mybir.dt.float32)
                    for c in range(K):
                        nc.tensor.matmul(
                            ps[:], idt_sb[:, :],
                            G[:, c * 128:c * 128 + d_hid + h],
                            start=(c == 0), stop=(c == K - 1))
                    post(b, ps)

            # ================= phase A: table 0 =================
            table_build(
                lambda b: xT_sb[:, b * 128:(b + 1) * 128],
                W0_sb, ad0_sb, tfull0)

            def bail():
                # drain something visible to out_d so the program stays valid
                t = stage.tile([128, n_cls], mybir.dt.float32, tag="t0")
                nc.vector.memset(t[:], 0.0)
                for b in range(nblk):
                    nc.sync.dma_start(out_d[b * 128:(b + 1) * 128, :], t[:])

            if stop_after == "A":
                bail()

            # ================= phase B: layer-0 edges =================
            def post0(b, ps):
                hv = h1_sb[:, b * d_hid:(b + 1) * d_hid]
                if ps is None:
                    nc.vector.tensor_copy(hv, b0_sb[:, :])
                    return
                z = small.tile([128, h], mybir.dt.float32, tag="z")
                nc.vector.tensor_scalar_add(z[:], ps[:, d_hid:d_hid + h],
                                            1e-16)
                iz = small.tile([128, h], mybir.dt.float32, tag="iz")
                nc.vector.reciprocal(iz[:], z[:])
                izb = iz[:, :].rearrange("p (j o) -> p j o",
                                         o=1).broadcast_to([128, h, dh])
                hv3 = hv.rearrange("p (j d) -> p j d", d=dh)
                nc.vector.tensor_mul(hv3, ps[:, 0:d_hid].rearrange(
                    "p (j d) -> p j d", d=dh), izb)
                nc.vector.tensor_add(hv, hv, b0_sb[:, :])

            if stop_after not in ("A",):
                edge_phase(tfull0, ad0_sb, post0)
            if stop_after == "B":
                bail()

            # ================= phase C: table 1 =================
            def lhsT1(b):
                pst = psT.tile([d_hid, 128], mybir.dt.float32, tag="ptr")
                nc.tensor.transpose(
                    pst[:], h1_sb[:, b * d_hid:(b + 1) * d_hid],
                    id32_sb[:, :])
                hT = stage.tile([d_hid, 128], mybir.dt.float32, tag="hT")
                nc.vector.tensor_copy(hT[:], pst[:])
                return hT[:, :]

            if stop_after not in ("A", "B"):
                table_build(lhsT1, W1_sb, ad1_sb, tfull1)
            if stop_after == "C":
                bail()

            # ================= phase D: layer-1 edges =================
            def post1(b, ps):
                hv = h2_sb[:, b * d_hid:(b + 1) * d_hid]
                if ps is None:
                    t = small.tile([128, d_hid], mybir.dt.float32, tag="t1")
                    nc.vector.tensor_copy(t[:], b1_sb[:, :])
                    nc.vector.tensor_scalar_max(hv, t[:], 0.0)
                    return
                z = small.tile([128, h], mybir.dt.float32, tag="z")
                nc.vector.tensor_scalar_add(z[:], ps[:, d_hid:d_hid + h],
                                            1e-16)
                iz = small.tile([128, h], mybir.dt.float32, tag="iz")
                nc.vector.reciprocal(iz[:], z[:])
                izb = iz[:, :].rearrange("p (j o) -> p j o",
                                         o=1).broadcast_to([128, h, dh])
                t = small.tile([128, d_hid], mybir.dt.float32, tag="t1")
                t3 = t[:, :].rearrange("p (j d) -> p j d", d=dh)
                nc.vector.tensor_mul(t3, ps[:, 0:d_hid].rearrange(
                    "p (j d) -> p j d", d=dh), izb)
                nc.vector.tensor_add(t[:], t[:], b1_sb[:, :])
                nc.vector.tensor_scalar_max(hv, t[:], 0.0)  # ReLU

            if stop_after not in ("A", "B", "C"):
                edge_phase(tfull1, ad1_sb, post1)
            if stop_after == "D":
                bail()

            # ================= phase E: output MLP + log_softmax ========
            skipE = stop_after in ("A", "B", "C", "D")
            for b in range(nblk if not skipE else 0):
                pst = psT.tile([d_hid, 128], mybir.dt.float32, tag="ptr")
                nc.tensor.transpose(
                    pst[:], h2_sb[:, b * d_hid:(b + 1) * d_hid],
                    id32_sb[:, :])
                hT = stage.tile([d_hid, 128], mybir.dt.float32, tag="hT")
                nc.vector.tensor_copy(hT[:], pst[:])
                po = psT.tile([128, n_cls], mybir.dt.float32, tag="po")
                nc.tensor.matmul(po[:], hT[:, :], Wo_sb[:, :], start=True,
                                 stop=True)
                t0 = stage.tile([128, n_cls], mybir.dt.float32, tag="t0")
                nc.vector.tensor_add(t0[:], po[:], bo_sb[:, :])
                m = small.tile([128, 1], mybir.dt.float32, tag="m")
                nc.vector.reduce_max(m[:], t0[:],
                                     axis=mybir.AxisListType.X)
                nc.vector.tensor_scalar(t0[:], t0[:], m[:, 0:1], None,
                                        op0=mybir.AluOpType.subtract)
                ex = stage.tile([128, n_cls], mybir.dt.float32, tag="ex")
                nc.scalar.activation(ex[:], t0[:],
                                     mybir.ActivationFunctionType.Exp)
                s = small.tile([128, 1], mybir.dt.float32, tag="s")
                nc.vector.reduce_sum(s[:], ex[:],
                                     axis=mybir.AxisListType.X)
                ls = small.tile([128, 1], mybir.dt.float32, tag="ls")
                nc.scalar.activation(ls[:], s[:],
                                     mybir.ActivationFunctionType.Ln)
                nc.vector.tensor_scalar(t0[:], t0[:], ls[:, 0:1], None,
                                        op0=mybir.AluOpType.subtract)
                nc.sync.dma_start(out_d[b * 128:(b + 1) * 128, :], t0[:])

    nc.compile()
    return nc


# ---------------------------------------------------------------- inputs
def make_in_maps(plan, inputs, f_in=F_IN, d_hid=D_HID, h=H, n_cls=N_CLS,
                 bf16=False):
    import ml_dtypes  # noqa: F401

    x = np.asarray(inputs["x"], dtype=np.float32)
    W0 = np.asarray(inputs["W0"], dtype=np.float32)
    W1 = np.asarray(inputs["W1"], dtype=np.float32)
    Wout = np.asarray(inputs["Wout"], dtype=np.float32)
    as0 = np.asarray(inputs["as0"], dtype=np.float32)
    ad0 = np.asarray(inputs["ad0"], dtype=np.float32)
    as1 = np.asarray(inputs["as1"], dtype=np.float32)
    ad1 = np.asarray(inputs["ad1"], dtype=np.float32)
    b0 = np.asarray(inputs["b0"], dtype=np.float32)
    b1 = np.asarray(inputs["b1"], dtype=np.float32)
    bout = np.asarray(inputs["bout"], dtype=np.float32)

    dh = d_hid // h
    npc, cores = plan["npc"], plan["cores"]
    old_of_new = plan["old_of_new"]

    def blockdiag(a):  # [h, dh] -> [d_hid, h]
        m = np.zeros((d_hid, h), dtype=np.float32)
        for j in range(h):
            m[j * dh:(j + 1) * dh, j] = a[j]
        return m

    W0cat = np.concatenate(
        [W0, W0 @ blockdiag(as0), W0 @ blockdiag(ad0)], axis=1)
    W1cat = np.concatenate(
        [W1, W1 @ blockdiag(as1), W1 @ blockdiag(ad1)], axis=1)

    tdt = ml_dtypes.bfloat16 if bf16 else np.float32
    ident = np.eye(128, dtype=tdt)
    ident32 = np.eye(128, dtype=np.float32)
    b0b = np.ascontiguousarray(np.broadcast_to(b0, (128, d_hid)))
    b1b = np.ascontiguousarray(np.broadcast_to(b1, (128, d_hid)))
    boutb = np.ascontiguousarray(np.broadcast_to(bout, (128, n_cls)))

    x_ext = np.zeros((npc * cores, f_in), dtype=np.float32)
    real = old_of_new >= 0
    x_ext[real] = x[old_of_new[real]]

    in_maps = []
    for c in range(cores):
        xs = x_ext[c * npc:(c + 1) * npc]
        m = dict(
            xT=np.ascontiguousarray(xs.T),
            W0cat=W0cat, W1cat=W1cat, Wout=Wout,
            b0b=b0b, b1b=b1b, boutb=boutb,
            identt=ident, ident32=ident32,
            idx_lo=np.ascontiguousarray(plan["idx_lo"][c])
            if plan["S_lo"] else np.zeros((128, 1), np.int16),
            idx_hi=np.ascontiguousarray(plan["idx_hi"][c])
            if plan["S_hi"] else np.zeros((128, 1), np.int16),
        )
        in_maps.append(m)
    return in_maps


def assemble_output(plan, results, n_cls=N_CLS):
    outs = np.concatenate([r["out"] for r in results], axis=0)
    return np.ascontiguousarray(outs[plan["new_of_old"]], dtype=np.float32)


# ---------------------------------------------------------------- entry
def kernel(**inputs):
    from concourse.bass_utils import run_bass_kernel_spmd

    edge_src = np.asarray(inputs["edge_src"]).astype(np.int64)
    edge_dst = np.asarray(inputs["edge_dst"]).astype(np.int64)

    bf16 = True
    key = (edge_src.tobytes(), edge_dst.tobytes(), bf16)
    kh = hash(key)
    if kh not in _CACHE:
        plan = make_plan(edge_src, edge_dst)
        nc = build_program(plan, bf16=bf16)
        _CACHE[kh] = (plan, nc)
    plan, nc = _CACHE[kh]

    in_maps = make_in_maps(plan, inputs, bf16=bf16)
    res = run_bass_kernel_spmd(nc, in_maps,
                               core_ids=list(range(plan["cores"])))
    return assemble_output(plan, res.results)
